# revision 33
# baseline (speedup 1.0000x reference)
"""CEAlignment Trainium2 kernel (8 NeuronCores, SPMD).

Sharding:
  - Phase 1 (MLPs): batch-data-parallel. Core c (c=0..7) runs MLP (c//4)
    [0 -> mlp1 on x1, 1 -> mlp2 on x2] on batch rows [(c%4)*128, +128).
    Activations stay batch-major [128, 2048] in SBUF; the stationary matmul
    operand is the transposed activation (PE transposes between layers; the
    initial x^T is prepared on host). Weights stream from HBM as the moving
    operand in float32r (FP22 multiply, fp32 accumulate, full PE rate at
    N=512). Biases are folded in as K=1 ones-row matmuls.
  - Phase 2: head_normalize (ddof=1) per 512-wide label block, transpose,
    then ONE AllToAll redistributes q^T so core c receives, at static
    addresses, label (c%4)'s full q1^T [512e x 512a] and q2^T [512e x 512b].
  - Phase 3: align = exp(q1_h @ q2_h^T / sqrt(E)) and a branchless Sinkhorn
    (2 unrolled iterations, convergence-flag blending identical to the
    reference's early-exit semantics; convergence checks use the squared
    form (d^2 <= ATOL^2) since |.| is not a DVE ALU op). Cross-partition
    sums use ones-column matmuls; partition broadcasts use gpsimd. Core c
    outputs the full [512, 512] matrix of label c%4 (cores 4-7 redundant).

W_MODE selects the weight-stream precision: "f32r" (fp32 weights, FP22
multiply — rel err ~5e-4, DMA-bound) or "bf16" (half the weight DMA,
hi/lo-split bf16 activations — rel err ~4.5e-3, ~25%% faster).
"""

import math
from contextlib import ExitStack

import numpy as np

import concourse.bacc as bacc
import concourse.bass as bass
import concourse.tile as tile
from concourse import mybir
from concourse.alu_op_type import AluOpType
from concourse.bass_utils import run_bass_kernel_spmd

# NOTE: TRN2 instructions may carry at most one sync wait; Bacc.compile()
# legalizes multi-wait instructions via generated event semaphores, so the
# program must be built on bacc.Bacc (not raw bass.Bass).

F32 = mybir.dt.float32
F32R = mybir.dt.float32r
AX = mybir.AxisListType.X
AF = mybir.ActivationFunctionType

B = 512          # batch (both sides)
D = 2048         # input dim
HD = 2048        # hidden dim
E = 512          # embed dim per label
L = 4            # num labels
R = 128          # batch rows per core
NCORES = 8
EPS = 1e-8
ATOL = 0.01
ISQ = 1.0 / math.sqrt(E)
SINKHORN_ITERS = 2

# moving-operand dtype for the MLP weight matmuls ("f32r" or "bf16")
W_MODE = "f32r"


def _r(ap):
    """Matmul-operand view (tiles are allocated as float32r already)."""
    return ap


def _emit(nc, tc, ctx, t):
    """Emit the SPMD program. `t` holds DRAM tensor handles."""
    w_dt = t["w0"].dtype  # weight dram dtype (f32 or bf16)

    def wview(ap):
        return ap

    NK = D // 128   # 16 k-tiles
    NN = HD // 512  # 4 n-tiles

    const_p = ctx.enter_context(tc.tile_pool(name="const", bufs=1))
    norm_p = ctx.enter_context(tc.tile_pool(name="norm", bufs=1))
    dram_p = ctx.enter_context(
        tc.tile_pool(name="dram", bufs=1, space=bass.MemorySpace.DRAM))
    ident = const_p.tile([128, 128], F32)
    nc.sync.dma_start(ident[:], t["ident"].ap())
    ones_sb = const_p.tile([128, 128], F32R)
    nc.sync.dma_start(ones_sb[:], t["ones"].ap())
    ones_row = ones_sb[0:1, :]
    ones_col = ones_sb[:, 0:1]
    p1m = const_p.tile([128, L], F32)   # p1 col for this core's label, tiled
    nc.sync.dma_start(p1m[:], t["p1m"].ap())
    p2r = const_p.tile([1, B], F32)     # p2 col for this core's label, row
    nc.sync.dma_start(p2r[:], t["p2r"].ap())
    epsb = const_p.tile([128, 1], F32)
    nc.vector.memset(epsb[:], EPS)

    qnT = norm_p.tile([128, L * E], F32R, tag="qnT")

    # ---------------- phase 1: MLP ----------------
    with ExitStack() as p1ctx:
        actT_p = p1ctx.enter_context(tc.tile_pool(name="actT", bufs=2))
        act_p = p1ctx.enter_context(tc.tile_pool(name="act", bufs=2))
        w_p = p1ctx.enter_context(tc.tile_pool(name="w", bufs=4))
        bias_p = p1ctx.enter_context(tc.tile_pool(name="bias", bufs=1))
        ps_mm = p1ctx.enter_context(
            tc.tile_pool(name="ps_mm", bufs=4, space=bass.MemorySpace.PSUM))
        ps_t = p1ctx.enter_context(
            tc.tile_pool(name="ps_t", bufs=2, space=bass.MemorySpace.PSUM))

        BF16 = mybir.dt.bfloat16
        if w_dt == BF16:
            # bf16 weights (half DMA) + hi/lo bf16 split of the stationary
            # activations (~16-bit effective mantissa, no DMA cost).
            actT_hi = actT_p.tile([128, D], BF16, tag="actT_hi")
            actT_lo = actT_p.tile([128, D], BF16, tag="actT_lo")
            nc.sync.dma_start(
                actT_hi[:].rearrange("p (j c) -> p j c", c=128),
                t["xT_hi"].ap().rearrange("(j p) c -> p j c", p=128))
            nc.sync.dma_start(
                actT_lo[:].rearrange("p (j c) -> p j c", c=128),
                t["xT_lo"].ap().rearrange("(j p) c -> p j c", p=128))
            actT_pair = (actT_hi, actT_lo)
        else:
            actT = actT_p.tile([128, D], F32R, tag="actT")
            nc.sync.dma_start(
                actT[:].rearrange("p (j c) -> p j c", c=128),
                t["xT"].ap().rearrange("(j p) c -> p j c", p=128))

        for lyr in range(4):
            act_out = act_p.tile([128, HD], F32, tag="act")
            w_dram = t[f"w{lyr}"].ap()
            bias_sb = bias_p.tile([1, HD], F32R, tag="bias")
            nc.sync.dma_start(bias_sb[:], t[f"b{lyr}"].ap())
            for n in range(NN):
                ps = ps_mm.tile([128, 512], F32, tag="mm")
                for kh in range(2):      # two half-k weight groups per n
                    wg = w_p.tile([128, 8 * 512], w_dt, tag="w")
                    nc.sync.dma_start(
                        wg[:].rearrange("p (k c) -> p k c", c=512),
                        w_dram[kh * 1024:(kh + 1) * 1024,
                               n * 512:(n + 1) * 512]
                        .rearrange("(k p) c -> p k c", p=128))
                    for kk in range(8):
                        k = kh * 8 + kk
                        wgk = wg[:, kk * 512:(kk + 1) * 512]
                        if w_dt == BF16:
                            nc.tensor.matmul(
                                ps[:], actT_hi[:, k * 128:(k + 1) * 128],
                                wgk, start=(k == 0), stop=False)
                            nc.tensor.matmul(
                                ps[:], actT_lo[:, k * 128:(k + 1) * 128],
                                wgk, start=False, stop=False)
                        else:
                            nc.tensor.matmul(
                                ps[:], _r(actT[:, k * 128:(k + 1) * 128]),
                                wgk, start=(k == 0), stop=False)
                # bias via K=1 matmul: ones_row^T @ b_row
                nc.tensor.matmul(
                    ps[:], _r(ones_row),
                    _r(bias_sb[0:1, n * 512:(n + 1) * 512]),
                    start=False, stop=True)
                if lyr < 3:
                    nc.scalar.activation(act_out[:, n * 512:(n + 1) * 512],
                                         ps[:], AF.Relu)
                else:
                    nc.scalar.copy(act_out[:, n * 512:(n + 1) * 512], ps[:])
            if lyr < 3:
                if w_dt == BF16:
                    actT_hi = actT_p.tile([128, D], BF16, tag="actT_hi")
                    actT_lo = actT_p.tile([128, D], BF16, tag="actT_lo")
                    for j in range(NK):
                        pt = ps_t.tile([128, 128], F32, tag="t")
                        nc.tensor.transpose(
                            pt[:], act_out[:, j * 128:(j + 1) * 128],
                            ident[:])
                        hi = actT_hi[:, j * 128:(j + 1) * 128]
                        nc.vector.tensor_copy(hi, pt[:])
                        nc.vector.tensor_tensor(
                            actT_lo[:, j * 128:(j + 1) * 128], pt[:], hi,
                            AluOpType.subtract)
                else:
                    actT_next = actT_p.tile([128, D], F32R, tag="actT")
                    for j in range(NK):
                        pt = ps_t.tile([128, 128], F32, tag="t")
                        nc.tensor.transpose(
                            pt[:], act_out[:, j * 128:(j + 1) * 128],
                            ident[:])
                        nc.vector.tensor_copy(
                            actT_next[:, j * 128:(j + 1) * 128], pt[:])
                    actT = actT_next
            else:
                q = act_out  # [128, 2048] = [128 rows, L*E]

        # ---------- phase 2a: head_normalize (ddof=1) + transpose ----------
        qn = norm_p.tile([128, L * E], F32, tag="qn")
        for h in range(L):
            blk = q[:, h * E:(h + 1) * E]
            ssum = norm_p.tile([128, 1], F32, tag="s1")
            nc.vector.reduce_sum(ssum[:], blk, axis=AX)
            nmean = norm_p.tile([128, 1], F32, tag="s2")
            nc.vector.tensor_scalar(nmean[:], ssum[:], -1.0 / E, None,
                                    AluOpType.mult)
            scr = norm_p.tile([128, E], F32, tag="scr")
            ss = norm_p.tile([128, 1], F32, tag="s3")
            nc.scalar.activation(scr[:], blk, AF.Square, bias=nmean[:],
                                 scale=1.0, accum_out=ss[:])
            # std = sqrt(ss/(E-1) + eps); rstd = 1/std (exact reciprocal)
            std = norm_p.tile([128, 1], F32, tag="s4")
            nc.scalar.activation(std[:], ss[:], AF.Sqrt, bias=epsb[:],
                                 scale=1.0 / (E - 1))
            rstd = norm_p.tile([128, 1], F32, tag="s5")
            nc.vector.reciprocal(rstd[:], std[:])
            nc.vector.tensor_scalar(qn[:, h * E:(h + 1) * E], blk, nmean[:],
                                    rstd[:], AluOpType.add, AluOpType.mult)

        for j in range(NK):
            pt = ps_t.tile([128, 128], F32, tag="t")
            nc.tensor.transpose(pt[:], qn[:, j * 128:(j + 1) * 128], ident[:])
            nc.vector.tensor_copy(qnT[:, j * 128:(j + 1) * 128], pt[:])

    # ---------- phase 2b: AllToAll ----------
    # Send buffer [2 * L*E, 128]: two stacked copies of q^T; chunk j
    # (rows [512j, 512j+512)) goes to rank j and is label (j%4)'s e-block.
    cc_in = dram_p.tile([2 * L * E, R], F32R, tag="cc_in")
    for rep in range(2):
        nc.sync.dma_start(
            cc_in[rep * L * E:(rep + 1) * L * E, :]
            .rearrange("(j r) c -> r j c", r=128),
            qnT[:].rearrange("p (j c) -> p j c", c=128))
    cc_out = dram_p.tile([NCORES * E, R], F32R, tag="cc_out")
    if nc.num_devices == 1:
        # single-core cost-model build: stand in for the AllToAll
        nc.sync.dma_start(cc_out[0:L * E, :], cc_in[0:L * E, :])
        nc.sync.dma_start(cc_out[L * E:2 * L * E, :], cc_in[0:L * E, :])
    else:
        nc.gpsimd.collective_compute(
            "AllToAll", AluOpType.bypass,
            replica_groups=[list(range(NCORES))],
            ins=[cc_in[:].opt()], outs=[cc_out[:].opt()])

    # ---------------- phase 3: alignment + sinkhorn ----------------
    snk_p = ctx.enter_context(tc.tile_pool(name="snk", bufs=1))
    ps_a = ctx.enter_context(
        tc.tile_pool(name="ps_a", bufs=4, space=bass.MemorySpace.PSUM))
    ps_s = ctx.enter_context(
        tc.tile_pool(name="ps_s", bufs=1, space=bass.MemorySpace.PSUM))
    ps_sc = ctx.enter_context(
        tc.tile_pool(name="ps_sc", bufs=2, space=bass.MemorySpace.PSUM))

    # q1T: lhsT tiles; [:, (4a+e)*128] = [e-tile of label, a-chunk a]
    q1T = snk_p.tile([128, 2048], F32R, tag="q1T")
    nc.sync.dma_start(
        q1T[:].rearrange("p (a e c) -> p a e c", a=4, e=4),
        cc_out[0:L * E, :].rearrange("(a e p) c -> p a e c", e=4, p=128))
    # q2T: rhs tiles; [:, e*512 + rb*128] = [e-tile, b-chunk rb]
    q2T = snk_p.tile([128, 2048], F32R, tag="q2T")
    for rb in range(4):
        nc.sync.dma_start(
            q2T[:].rearrange("p (e rb c) -> p e rb c", e=4, rb=4)[:, :, rb],
            cc_out[(4 + rb) * E:(5 + rb) * E, :]
            .rearrange("(e p) c -> p e c", p=128))

    # align: A = exp((q1_h @ q2_h^T) / sqrt(E)); 4 a-tiles in one wide tile
    cur = snk_p.tile([128, 2048], F32R, tag="cur0")
    for a in range(4):
        ps = ps_a.tile([128, 512], F32, tag="al")
        for e in range(4):
            nc.tensor.matmul(
                ps[:], _r(q1T[:, (4 * a + e) * 128:(4 * a + e + 1) * 128]),
                _r(q2T[:, e * 512:(e + 1) * 512]),
                start=(e == 0), stop=(e == 3))
        nc.scalar.activation(cur[:, a * 512:(a + 1) * 512], ps[:], AF.Exp,
                             scale=ISQ)

    done_prev = None   # [1,1] flag: converged in an earlier iteration
    prev_out = None    # wide tile to keep if done_prev
    for it in range(SINKHORN_ITERS):
        # ---- column normalize: m1 = cur * (p2 / (colsum + eps)) ----
        pc = ps_s.tile([1, 512], F32, tag="cs")
        for a in range(4):
            nc.tensor.matmul(pc[:], _r(ones_col),
                             _r(cur[:, a * 512:(a + 1) * 512]),
                             start=(a == 0), stop=(a == 3))
        cse = snk_p.tile([1, 512], F32, tag="cse")
        nc.vector.tensor_scalar(cse[:], pc[:], EPS, None, AluOpType.add)
        csr = snk_p.tile([1, 512], F32, tag="csr")
        nc.vector.reciprocal(csr[:], cse[:])
        srow = snk_p.tile([1, 512], F32, tag="srow")
        nc.vector.tensor_tensor(srow[:], csr[:], p2r[:], AluOpType.mult)
        sful = snk_p.tile([128, 512], F32, tag="sful")
        nc.gpsimd.partition_broadcast(sful[:], srow[:])
        m1 = snk_p.tile([128, 2048], F32, tag="m1")
        rs4 = snk_p.tile([128, 4], F32, tag="rs4")
        for a in range(4):
            nc.vector.scalar_tensor_tensor(
                m1[:, a * 512:(a + 1) * 512], cur[:, a * 512:(a + 1) * 512],
                1.0, sful[:], AluOpType.mult, AluOpType.mult,
                accum_out=rs4[:, a:a + 1])
        # ---- row_ok: all (rowsum(m1) - p1)^2 <= ATOL^2 ----
        dev4 = snk_p.tile([128, 4], F32, tag="dev4")
        nc.vector.tensor_tensor(dev4[:], rs4[:], p1m[:], AluOpType.subtract)
        dev4sq = snk_p.tile([128, 4], F32, tag="dev4sq")
        nc.vector.tensor_tensor(dev4sq[:], dev4[:], dev4[:], AluOpType.mult)
        dev4r = snk_p.tile([128, 4], F32R, tag="dev4r")
        nc.vector.tensor_scalar(dev4r[:], dev4sq[:], ATOL * ATOL, 0.0,
                                AluOpType.subtract, AluOpType.max)
        pv = ps_sc.tile([1, 4], F32, tag="tiny")
        nc.tensor.matmul(pv[:], _r(ones_col), _r(dev4r[:]),
                         start=True, stop=True)
        vrow = snk_p.tile([1, 1], F32, tag="vrow")
        s14 = snk_p.tile([1, 4], F32, tag="s14")
        nc.vector.tensor_scalar(s14[:], pv[:], 0.0, None, AluOpType.add,
                                AluOpType.add, accum_out=vrow[:])
        grow = snk_p.tile([1, 1], F32, tag="grow")
        nc.vector.tensor_scalar(grow[:], vrow[:], 1e-30, None,
                                AluOpType.is_le)
        # ---- row normalize: m2 = m1 * (p1 / (rowsum + eps)) ----
        re4 = snk_p.tile([128, 4], F32, tag="re4")
        nc.vector.tensor_scalar(re4[:], rs4[:], EPS, None, AluOpType.add)
        rr4 = snk_p.tile([128, 4], F32, tag="rr4")
        nc.vector.reciprocal(rr4[:], re4[:])
        f4 = snk_p.tile([128, 4], F32, tag="f4")
        nc.vector.tensor_tensor(f4[:], rr4[:], p1m[:], AluOpType.mult)
        m2 = snk_p.tile([128, 2048], F32R, tag="m2")
        for a in range(4):
            nc.vector.tensor_scalar(m2[:, a * 512:(a + 1) * 512],
                                    m1[:, a * 512:(a + 1) * 512],
                                    f4[:, a:a + 1], None, AluOpType.mult)
        # ---- col_ok: all (colsum(m2) - p2)^2 <= ATOL^2 ----
        pc2 = ps_s.tile([1, 512], F32, tag="cs")
        for a in range(4):
            nc.tensor.matmul(pc2[:], _r(ones_col),
                             _r(m2[:, a * 512:(a + 1) * 512]),
                             start=(a == 0), stop=(a == 3))
        cd = snk_p.tile([1, 512], F32, tag="cd")
        nc.vector.tensor_tensor(cd[:], pc2[:], p2r[:], AluOpType.subtract)
        cd2 = snk_p.tile([1, 512], F32, tag="cd2")
        nc.vector.tensor_tensor(cd2[:], cd[:], cd[:], AluOpType.mult)
        cda = snk_p.tile([1, 512], F32, tag="cda")
        nc.vector.tensor_scalar(cda[:], cd2[:], ATOL * ATOL, None,
                                AluOpType.subtract)
        vcol = snk_p.tile([1, 1], F32, tag="vcol")
        cdr = snk_p.tile([1, 512], F32, tag="cdr")
        nc.vector.tensor_scalar(cdr[:], cda[:], 0.0, None, AluOpType.max,
                                AluOpType.add, accum_out=vcol[:])
        gcol = snk_p.tile([1, 1], F32, tag="gcol")
        nc.vector.tensor_scalar(gcol[:], vcol[:], 1e-30, None,
                                AluOpType.is_le)
        # ---- new = grow ? m1 : m2 (branchless, full-width ops) ----
        pg = snk_p.tile([128, 1], F32, tag="pg")
        nc.gpsimd.partition_broadcast(pg[:], grow[:])
        d = snk_p.tile([128, 2048], F32, tag="d")
        nc.vector.tensor_tensor(d[:], m1[:], m2[:], AluOpType.subtract)
        nw = snk_p.tile([128, 2048], F32R, tag=f"nw{it}")
        nc.vector.scalar_tensor_tensor(nw[:], d[:], pg[:], m2[:],
                                       AluOpType.mult, AluOpType.add)
        if it == 0:
            done_prev = snk_p.tile([1, 1], F32, tag="done")
            nc.vector.tensor_tensor(done_prev[:], grow[:], gcol[:],
                                    AluOpType.max)
            prev_out = nw
            cur = nw
        else:
            # final = done_prev ? prev_out : new
            pd = snk_p.tile([128, 1], F32, tag="pd")
            nc.gpsimd.partition_broadcast(pd[:], done_prev[:])
            d2 = snk_p.tile([128, 2048], F32, tag="d2")
            nc.vector.tensor_tensor(d2[:], prev_out[:], nw[:],
                                    AluOpType.subtract)
            fin = snk_p.tile([128, 2048], F32, tag="fin")
            nc.vector.scalar_tensor_tensor(fin[:], d2[:], pd[:], nw[:],
                                           AluOpType.mult, AluOpType.add)
            cur = fin

    # out[a*128 + r, c] = cur[r, a*512 + c] -- one DMA
    nc.sync.dma_start(
        t["out"].ap().rearrange("(a r) c -> r a c", r=128),
        cur[:].rearrange("p (a c) -> p a c", c=512))


def build_program(w_mode=W_MODE, num_devices=NCORES):
    w_dt = F32R if w_mode == "f32r" else mybir.dt.bfloat16
    nc = bacc.Bacc("TRN2", target_bir_lowering=False, debug=False,
                   num_devices=num_devices)
    t = {}
    if w_mode == "bf16":
        t["xT_hi"] = nc.dram_tensor("xT_hi", [D, R], mybir.dt.bfloat16,
                                    kind="ExternalInput")
        t["xT_lo"] = nc.dram_tensor("xT_lo", [D, R], mybir.dt.bfloat16,
                                    kind="ExternalInput")
    else:
        t["xT"] = nc.dram_tensor("xT", [D, R], F32R, kind="ExternalInput")
    for lyr in range(4):
        t[f"w{lyr}"] = nc.dram_tensor(f"w{lyr}", [D, HD], w_dt,
                                      kind="ExternalInput")
        t[f"b{lyr}"] = nc.dram_tensor(f"b{lyr}", [1, HD], F32R,
                                      kind="ExternalInput")
    t["p1m"] = nc.dram_tensor("p1m", [128, L], F32, kind="ExternalInput")
    t["p2r"] = nc.dram_tensor("p2r", [1, B], F32, kind="ExternalInput")
    t["ident"] = nc.dram_tensor("ident", [128, 128], F32,
                                kind="ExternalInput")
    t["ones"] = nc.dram_tensor("ones", [128, 128], F32R,
                               kind="ExternalInput")
    t["out"] = nc.dram_tensor("out", [B, B], F32, kind="ExternalOutput")

    with ExitStack() as ctx:
        tc = ctx.enter_context(tile.TileContext(nc))
        _emit(nc, tc, ctx, t)
    nc.compile()
    return nc


def make_in_maps(x1, x2, x1_probs, x2_probs, mlp1_ws, mlp1_bs, mlp2_ws,
                 mlp2_bs, w_mode=W_MODE):
    if w_mode == "f32r":
        w_np = np.float32
    else:
        import ml_dtypes
        w_np = ml_dtypes.bfloat16
    xT = [np.ascontiguousarray(np.asarray(x1, np.float32).T),
          np.ascontiguousarray(np.asarray(x2, np.float32).T)]
    ws = [[np.ascontiguousarray(np.asarray(w, np.float32).astype(w_np))
           for w in mlp1_ws],
          [np.ascontiguousarray(np.asarray(w, np.float32).astype(w_np))
           for w in mlp2_ws]]
    bs = [[np.asarray(b, np.float32).reshape(1, HD) for b in mlp1_bs],
          [np.asarray(b, np.float32).reshape(1, HD) for b in mlp2_bs]]
    p1 = np.asarray(x1_probs, np.float32)
    p2 = np.asarray(x2_probs, np.float32)
    ident = np.eye(128, dtype=np.float32)
    in_maps = []
    for c in range(NCORES):
        m = c // 4          # which MLP
        rslice = c % 4      # which batch rows
        h = c % 4           # which label for sinkhorn
        xTc = np.ascontiguousarray(xT[m][:, rslice * R:(rslice + 1) * R])
        if w_mode == "bf16":
            import ml_dtypes
            hi = xTc.astype(ml_dtypes.bfloat16)
            lo = (xTc - hi.astype(np.float32)).astype(ml_dtypes.bfloat16)
            xin = {"xT_hi": hi, "xT_lo": lo}
        else:
            xin = {"xT": xTc}
        d = {**xin,
             "p1m": np.ascontiguousarray(p1[:, h].reshape(4, 128).T),
             "p2r": np.ascontiguousarray(p2[:, h].reshape(1, B)),
             "ident": ident,
             "ones": np.ones((128, 128), np.float32)}
        for lyr in range(4):
            d[f"w{lyr}"] = ws[m][lyr]
            d[f"b{lyr}"] = bs[m][lyr]
        in_maps.append(d)
    return in_maps


_PROGRAM_CACHE = {}


def kernel(x1, x2, x1_probs, x2_probs, mlp1_ws, mlp1_bs, mlp2_ws, mlp2_bs,
           **run_kwargs):
    if W_MODE not in _PROGRAM_CACHE:
        _PROGRAM_CACHE[W_MODE] = build_program(W_MODE)
    nc = _PROGRAM_CACHE[W_MODE]
    in_maps = make_in_maps(x1, x2, x1_probs, x2_probs, mlp1_ws, mlp1_bs,
                           mlp2_ws, mlp2_bs)
    res = run_bass_kernel_spmd(nc, in_maps, core_ids=list(range(NCORES)),
                               **run_kwargs)
    out = np.stack([res.results[h]["out"] for h in range(L)], axis=2)
    kernel.last_results = res
    return np.ascontiguousarray(out.astype(np.float32))
